# revision 15
# baseline (speedup 1.0000x reference)
"""Two-branch attention kernel for Trainium2 (8 NeuronCores, batch-parallel).

out1 = proj(softmax(q k^T / 8) v),  out2 = proj(softmax(q k2^T / 8) v2)
with q,k,v from x and k2,v2 from x2 (q shared across branches).

Sharding: batch dim (8) -> one batch element per core. No collectives.

v2 design (vs 355us slotted baseline):
  * slot-level software pipeline: 48 slots (12 attention units x 4 kj-pairs),
    each slot = AV(u,k) + filler + S/exp(u+1,k).  ALL non-attention matmul
    work (QKV for both inputs, both branches' projections) is scheduled by a
    deadline-driven filler queue so every slot carries enough independent PE
    work to cover the ACT exp latency (the attention-phase pacer) and the PE
    never idles / drops out of its max p-state.
  * lead-in shrunk to the minimal prefix (q/k group 0-1, v tiles 0-3,
    S/exp for unit 0); the rest of QKV-x rides the filler queue.
  * AV accumulators split per (head, column-half) into 1-bank PSUM pieces:
    finer eviction, earlier bank recycle at unit boundaries.
  * softmax row-sums DMA'd directly from PSUM to the DRAM-bounce gather (no
    [1,1024] DVE copies); all-f32 r path; batched reciprocal as before.
  * tail: branch-1 projections for qi 0-3 run as g0-g4 partials inside unit
    11 (which has no next-unit S work), finished with the g5 contribution
    after the last normalize; only qi 4-7 remain as full tail projections.
  * contiguous host-side weight layouts (one DMA per tensor, 128-col chunks
    for wqk in consumption order) so the first matmul starts ~3us earlier.
  * bf16 output (host converts to f32): halves the output DMA flush.
"""
import sys
for _p in ('/opt/trn_rl_repo',):
    if _p not in sys.path:
        sys.path.insert(0, _p)

import numpy as np

MODE = 'v2-slotted'

B, N, D, H, HD = 8, 1024, 768, 12, 64
SCALE = HD ** -0.5
NDT = D // 128       # 6 dim tiles
NQT = N // 128       # 8 token tiles
P = 128
AUG = HD + 1         # 65: head dim + ones column for row sums
NU = 12              # (branch, g) attention units


# ----------------------------------------------------------------------------
# workaround: walrus rejects >2 sem waits on one instruction; TileContext's
# tail drain carries one wait per active logical proc. Split them across
# single-wait SP nops and emit a bare drain.
def _install_tilefix():
    import bass_rust
    import concourse.tile as tile

    def _drain_and_barrier_split(self, tick_clock, wait_clock):
        gc = tick_clock.global_clock
        ticks = [gc[i] for i in range(27)]
        for i, t in enumerate(ticks):
            if t > 0:
                vc = bass_rust.VectorClock(
                    [t if j == i else 0 for j in range(len(ticks))])
                nop = self.nc.sync.nop()
                wait_clock.add_sem_waits(
                    nop.ins, bass_rust.ScopedClock({None: vc}))
        self.nc.sync.drain()
        self.nc.all_engine_barrier()
        assert self.sems is not None
        popped = self.nc._tile_sem_poison_stack.pop()
        assert popped is self._sem_poison
        self.nc.clear_and_free_semaphores(list(self.sems.allocated().values()))
        self.nc.all_engine_barrier()

    tile.TileContext._drain_and_barrier = _drain_and_barrier_split


def _split_multiwaits(nc, max_waits=1):
    """walrus codegen rejects instructions carrying more than `max_waits`
    sync waits; hoist the extras onto same-engine nops placed just before."""
    import bass_rust
    import concourse.mybir as mybir
    cnt = 0
    for bb in nc.main_func.blocks:
        insts = bb.instructions
        i = 0
        while i < len(insts):
            ins = insts[i]
            si = getattr(ins, 'sync_info', None)
            if si is not None and si.on_wait and len(si.on_wait) > max_waits:
                waits = list(si.on_wait)
                extras, keep = waits[:-max_waits], waits[-max_waits:]
                for w in extras:
                    nop = mybir.InstNoOp(name=f"I-swx{cnt}", ins=[], outs=[])
                    cnt += 1
                    nop.engine = ins.engine
                    nop.sync_info = bass_rust.SyncInfo(on_wait=[w],
                                                       on_update=[])
                    insts.insert(i, nop)
                    i += 1
                ins.sync_info = bass_rust.SyncInfo(
                    on_wait=keep, on_update=list(si.on_update))
            i += 1
    return cnt


_built = None


def _build(split=True):
    """Build the SPMD bass program once. Returns (nc, n_split_waits).

    split=False skips the multiwait-splitting pass (CoreSim chokes on the
    synthesized no-update nops; walrus/HW requires them)."""
    global _built
    if _built is not None:
        return _built
    _install_tilefix()
    from contextlib import ExitStack
    import concourse.bass as bass
    import concourse.tile as tile
    from concourse import mybir

    dt = mybir.dt
    mdt = dt.bfloat16          # matmul operand dtype everywhere

    nc = bass.Bass("TRN2", target_bir_lowering=False, debug=False,
                   num_devices=8)

    # DRAM I/O (per core); weight layouts are pre-transposed on host so every
    # load is contiguous per partition.
    xt_d = nc.dram_tensor("xt", [P, NDT, N], mdt, kind="ExternalInput")
    x2t_d = nc.dram_tensor("x2t", [P, NDT, N], mdt, kind="ExternalInput")
    # wqk[p, o, i, c]: output col block o (0-5 q, 6-11 k), contraction tile i
    wqk_d = nc.dram_tensor("wqk", [P, 2 * NDT, NDT, P], mdt,
                           kind="ExternalInput")
    wv_d = nc.dram_tensor("wv", [P, NDT, D], mdt, kind="ExternalInput")
    wp_d = nc.dram_tensor("wp", [P, NDT, D], mdt, kind="ExternalInput")
    bias_d = nc.dram_tensor("bias", [P, D], mdt, kind="ExternalInput")
    out_d = nc.dram_tensor("out", [2, N, D], mdt, kind="ExternalOutput")

    units = [(0, g) for g in range(NDT)] + [(1, g) for g in range(NDT)]
    BATCHES = {0: [(0, 1, 2), (3, 4, 5)],
               1: [(0, 1, 2), (3,), (4,), (5,)]}
    G2B = {br: {g: (bi, list(gs).index(g))
                for bi, gs in enumerate(BATCHES[br]) for g in gs}
           for br in (0, 1)}

    with tile.TileContext(nc) as tc, ExitStack() as top:
        # PSUM budget (8 banks): pp tag "S" = 2 x [P,2,512] f32 (4 banks,
        # S tiles + every filler's psum); po tag "O" = 4 x [P,512] f32
        # (4 banks, AV accumulator pieces + tail partial pieces).
        pp = top.enter_context(tc.tile_pool(name="ps", bufs=2, space="PSUM"))
        po = top.enter_context(tc.tile_pool(name="po", bufs=4, space="PSUM"))
        dram_rb = top.enter_context(tc.tile_pool(name="dram_rb", bufs=8,
                                                 space="DRAM"))
        persist = top.enter_context(tc.tile_pool(name="persist", bufs=1))
        pool_pt = top.enter_context(tc.tile_pool(name="pt", bufs=5))
        pool_sm = top.enter_context(tc.tile_pool(name="sm", bufs=2))
        pool_rv = top.enter_context(tc.tile_pool(name="rv", bufs=1))
        pool_res = top.enter_context(tc.tile_pool(name="res", bufs=2))
        pha = top.enter_context(tc.tile_pool(name="pha", bufs=1))

        # persistent SBUF tiles (bf16)
        qT = persist.tile([P, NDT, N], mdt, tag="qT")
        kT1 = persist.tile([P, NDT, N], mdt, tag="kT1")
        kT2 = persist.tile([P, NDT, N], mdt, tag="kT2")
        vaug1 = persist.tile([P, NQT, H * AUG], mdt, tag="va1")
        vaug2 = persist.tile([P, NQT, H * AUG], mdt, tag="va2")
        wp_t = persist.tile([P, NDT, D], mdt, tag="wp")
        bias_t = persist.tile([P, D], mdt, tag="bias")
        # SBUF-resident br1 proj partials for qi 4-6 (bias folded in)
        psb = persist.tile([P, 3, D], mdt, tag="psb")
        ot = [persist.tile([P, NDT, N], mdt, tag=f"ot{b}", name=f"ot{b}")
              for b in (0, 1)]
        # row-sum gather targets: [16*len(gs), 128] bf16, filled via a DRAM
        # bounce (partition reshape) from a per-unit staging row that the
        # (idle) Pool engine copies out of the PSUM r-rows.
        rall = {(br, bi): persist.tile([16 * len(gs), P], mdt,
                                       tag=f"rall{br}{bi}",
                                       name=f"rall{br}{bi}")
                for br in (0, 1) for bi, gs in enumerate(BATCHES[br])}

        # phase-A inputs
        xt_t = pha.tile([P, NDT, N], mdt, tag="xt")
        x2t_t = pha.tile([P, NDT, N], mdt, tag="x2t")
        wqk_t = pha.tile([P, 2 * NDT, NDT, P], mdt, tag="wqk")
        wv_t = pha.tile([P, NDT, D], mdt, tag="wv")

        # ones columns of vaug via memset on the idle Pool engine (DMAs here
        # would serialize the SP for ~20us of 2-byte-element descriptors)
        for va in (vaug1, vaug2):
            nc.gpsimd.memset(
                va.rearrange("p t (h e) -> p t h e",
                             e=AUG)[:, :, :, HD:AUG], 1.0)

        # input DMAs in consumption order; wqk per 128-col block
        def dma_wqk(o):
            nc.sync.dma_start(out=wqk_t[:, o], in_=wqk_d[:, o])
        dma_wqk(0)                                   # q g0
        for i in range(NDT):
            nc.sync.dma_start(out=xt_t[:, i, :], in_=xt_d[:, i, :])
        dma_wqk(NDT)                                 # k g0
        dma_wqk(1)                                   # q g1
        dma_wqk(NDT + 1)                             # k g1
        nc.sync.dma_start(out=wv_t, in_=wv_d[:])
        dma_wqk(2)
        dma_wqk(NDT + 2)
        nc.sync.dma_start(out=x2t_t, in_=x2t_d[:])
        for o in (3, NDT + 3, 4, NDT + 4, 5, NDT + 5):
            dma_wqk(o)
        nc.sync.dma_start(out=wp_t, in_=wp_d[:])
        nc.sync.dma_start(out=bias_t, in_=bias_d[:])

        # ---------------- QKV emit units --------------------------------
        def qkT_group(src_x, o_block, dst, g):
            """one [128,1024] output tile of q^T/k^T via W-stationary."""
            psf = pp.tile([P, 2, 512], dt.float32, tag="S")
            ps = psf.rearrange("p a n -> p (a n)")
            for i in range(NDT):
                wsl = wqk_t[:, o_block, i, :]
                for c in range(2):
                    nc.tensor.matmul(
                        ps[:, c * 512:(c + 1) * 512], wsl,
                        src_x[:, i, c * 512:(c + 1) * 512],
                        start=(i == 0), stop=(i == NDT - 1))
            nc.vector.tensor_copy(dst[:, g, :], ps[:])

        def v_group(src_x, vaug_t, t):
            """one [128tok, 768] v tile via x-stationary into vaug."""
            psf = pp.tile([P, 2, 512], dt.float32, tag="S")
            ps = psf.rearrange("p a n -> p (a n)")
            for i in range(NDT):
                xsl = src_x[:, i, t * P:(t + 1) * P]
                for c0, cn in ((0, 512), (512, 256)):
                    nc.tensor.matmul(
                        ps[:, c0:c0 + cn], xsl, wv_t[:, i, c0:c0 + cn],
                        start=(i == 0), stop=(i == NDT - 1))
            src = ps[:, 0:D].rearrange("p (h e) -> p h e", e=HD)
            dst = vaug_t[:, t, :].rearrange("p (h e) -> p h e",
                                            e=AUG)[:, :, 0:HD]
            nc.vector.tensor_copy(dst, src)

        # ---------------- attention units -------------------------------
        kTs, vas = (kT1, kT2), (vaug1, vaug2)
        pt_tiles = {}   # (u, kjp) -> tile [P, 2, 2, N] (hh, kjl, qi)
        po_pieces = {}  # u -> [4 tiles [P,512]] indexed hh*2+c

        def part1(u, kjp):
            """S + exp for kj pair kjp of unit u -> pt tile (bf16)."""
            br, g = units[u]
            kT_t = kTs[br]
            pt = pool_pt.tile([P, 2, 2, N], mdt, tag="pt")
            pt_tiles[(u, kjp)] = pt
            for kjl in range(2):
                kj = 2 * kjp + kjl
                for c in range(2):
                    sc = pp.tile([P, 2, 512], dt.float32, tag="S")
                    for hh in range(2):
                        r0 = hh * HD
                        nc.tensor.matmul(
                            sc[:, hh, :],
                            kT_t[r0:r0 + HD, g, kj * P:(kj + 1) * P],
                            qT[r0:r0 + HD, g, c * 512:(c + 1) * 512],
                            start=True, stop=True, skip_group_check=True)
                    nc.scalar.activation(
                        pt[:, :, kjl, c * 512:(c + 1) * 512], sc[:],
                        mybir.ActivationFunctionType.Exp, scale=SCALE)

        def emit_av(u, kjp):
            br, g = units[u]
            va = vas[br]
            pt = pt_tiles[(u, kjp)]
            if kjp == 0:
                po_pieces[u] = [po.tile([P, 512], dt.float32, tag="O",
                                        name=f"po{u}_{hc}")
                                for hc in range(4)]
            pcs = po_pieces[u]
            for kjl in range(2):
                kj = 2 * kjp + kjl
                for hh in range(2):
                    h = 2 * g + hh
                    for c in range(2):
                        nc.tensor.matmul(
                            pcs[hh * 2 + c][0:AUG, :],
                            va[:, kj, h * AUG:(h + 1) * AUG],
                            pt[:, hh, kjl, c * 512:(c + 1) * 512],
                            start=(kj == 0), stop=(kj == NQT - 1),
                            skip_group_check=True)

        def evict_unit(u):
            """evict AV pieces: o rows (unnormalized) on DVE, r rows staged
            by the (idle) Pool engine into one [1,2048] row then DMA-gathered.
            The last unit evicts o on ACT (idle there) to unload DVE."""
            br, g = units[u]
            bi, j = G2B[br][g]
            cp = nc.scalar.copy if u == NU - 1 else nc.vector.tensor_copy
            pcs = po_pieces[u]
            rt = pool_sm.tile([1, 4 * 512], mdt, tag="rt", bufs=1)
            for hh in range(2):
                for c in range(2):
                    piece = pcs[hh * 2 + c]
                    hc = hh * 2 + c
                    nc.vector.tensor_copy(rt[:, hc * 512:(hc + 1) * 512],
                                          piece[HD:HD + 1, :])
                    cp(ot[br][hh * HD:(hh + 1) * HD, g,
                              c * 512:(c + 1) * 512], piece[0:HD, :])
            rw = dram_rb.tile([16, P], mdt, tag="rw")
            nc.sync.dma_start(out=rw[:], in_=rt[:])
            nc.sync.dma_start(out=rall[(br, bi)][16 * j:16 * j + 16, :],
                              in_=rw[:])

        def norm_batch(br, bi):
            """batched 1/r (partition-parallel) + broadcast + in-place scale."""
            gs = BATCHES[br][bi]
            rinv = pool_rv.tile([16 * len(gs), P], dt.float32, tag="rinv",
                                padded_shape=[48, P])
            nc.vector.reciprocal(rinv[:], rall[(br, bi)][:])
            rd = dram_rb.tile([2 * len(gs), N], dt.float32, tag="rd",
                              padded_shape=[6, N])
            nc.sync.dma_start(out=rd[:], in_=rinv[:])
            for jj, g in enumerate(gs):
                rb = pool_sm.tile([P, N], dt.float32, tag="rb")
                # 32-partition slices: partition_broadcast is DMA-descriptor
                # bound (~1/partition), so split across 4 parallel queues
                for q in range(4):
                    nc.sync.dma_start(
                        out=rb[q * 32:(q + 1) * 32, :],
                        in_=rd[2 * jj + q // 2, :].partition_broadcast(32))
                sl = ot[br][:, g, :]
                nc.vector.tensor_tensor(sl, sl, rb[:],
                                        mybir.AluOpType.mult)

        # ---------------- projections -----------------------------------
        def bias_out(br, qi, ps):
            res = pool_res.tile([P, D], mdt, tag="res")
            nc.vector.tensor_add(res[:], ps[:], bias_t[:])
            nc.sync.dma_start(out=out_d[br, qi * P:(qi + 1) * P, :],
                              in_=res[:])

        def proj_qi(br, qi):
            psf = pp.tile([P, 2, 512], dt.float32, tag="S")
            ps = psf.rearrange("p a n -> p (a n)")[:, 0:D]
            for g in range(NDT):
                osl = ot[br][:, g, qi * P:(qi + 1) * P]
                for c0, cn in ((0, 512), (512, 256)):
                    nc.tensor.matmul(
                        ps[:, c0:c0 + cn], osl, wp_t[:, g, c0:c0 + cn],
                        start=(g == 0), stop=(g == NDT - 1),
                        skip_group_check=True)
            bias_out(br, qi, ps)

        partials = {}   # qi -> ('S', ps) | ('O', piece_a, piece_b)

        def partial_S(qi):
            """br1 proj partial over g0..g4 into a pp tag-S tile."""
            psf = pp.tile([P, 2, 512], dt.float32, tag="S")
            ps = psf.rearrange("p a n -> p (a n)")[:, 0:D]
            for g in range(NDT - 1):
                osl = ot[1][:, g, qi * P:(qi + 1) * P]
                for c0, cn in ((0, 512), (512, 256)):
                    nc.tensor.matmul(
                        ps[:, c0:c0 + cn], osl, wp_t[:, g, c0:c0 + cn],
                        start=(g == 0), stop=False, skip_group_check=True)
            partials[qi] = ('S', ps)

        def partial_O(qi):
            """br1 proj partial over g0..g4 into two po tag-O pieces."""
            pa = po.tile([P, 512], dt.float32, tag="O", name=f"pa{qi}")
            pb = po.tile([P, 512], dt.float32, tag="O", name=f"pb{qi}")
            for g in range(NDT - 1):
                osl = ot[1][:, g, qi * P:(qi + 1) * P]
                nc.tensor.matmul(pa[:, :], osl, wp_t[:, g, 0:512],
                                 start=(g == 0), stop=False,
                                 skip_group_check=True)
                nc.tensor.matmul(pb[:, 0:256], osl, wp_t[:, g, 512:768],
                                 start=(g == 0), stop=False,
                                 skip_group_check=True)
            partials[qi] = ('O', pa, pb)

        def partial_sb(qi):
            """br1 proj partial over g0..g4, evicted (+bias) to SBUF bf16 on
            the Pool engine so the PSUM bank recycles immediately."""
            psf = pp.tile([P, 2, 512], dt.float32, tag="S")
            ps = psf.rearrange("p a n -> p (a n)")[:, 0:D]
            for g in range(NDT - 1):
                osl = ot[1][:, g, qi * P:(qi + 1) * P]
                for c0, cn in ((0, 512), (512, 256)):
                    nc.tensor.matmul(
                        ps[:, c0:c0 + cn], osl, wp_t[:, g, c0:c0 + cn],
                        start=(g == 0), stop=(g == NDT - 2),
                        skip_group_check=True)
            nc.vector.scalar_tensor_tensor(
                psb[:, qi - 4, :], ps[:], 1.0, bias_t[:],
                mybir.AluOpType.mult, mybir.AluOpType.add)

        def finish_sb(qi):
            """g5 contribution + SBUF partial + (already-folded) bias."""
            psf = pp.tile([P, 2, 512], dt.float32, tag="S")
            ps = psf.rearrange("p a n -> p (a n)")[:, 0:D]
            g = NDT - 1
            osl = ot[1][:, g, qi * P:(qi + 1) * P]
            for c0, cn in ((0, 512), (512, 256)):
                nc.tensor.matmul(ps[:, c0:c0 + cn], osl,
                                 wp_t[:, g, c0:c0 + cn],
                                 start=True, stop=True,
                                 skip_group_check=True)
            res = pool_res.tile([P, D], mdt, tag="res")
            nc.vector.scalar_tensor_tensor(
                res[:], ps[:], 1.0, psb[:, qi - 4, :],
                mybir.AluOpType.mult, mybir.AluOpType.add)
            nc.sync.dma_start(out=out_d[1, qi * P:(qi + 1) * P, :],
                              in_=res[:])

        def finish_qi(qi):
            g = NDT - 1
            osl = ot[1][:, g, qi * P:(qi + 1) * P]
            ent = partials[qi]
            if ent[0] == 'S':
                ps = ent[1]
                for c0, cn in ((0, 512), (512, 256)):
                    nc.tensor.matmul(
                        ps[:, c0:c0 + cn], osl, wp_t[:, g, c0:c0 + cn],
                        start=False, stop=True, skip_group_check=True)
                bias_out(1, qi, ps)
            else:
                _, pa, pb = ent
                nc.tensor.matmul(pa[:, :], osl, wp_t[:, g, 0:512],
                                 start=False, stop=True,
                                 skip_group_check=True)
                nc.tensor.matmul(pb[:, 0:256], osl, wp_t[:, g, 512:768],
                                 start=False, stop=True,
                                 skip_group_check=True)
                res = pool_res.tile([P, D], mdt, tag="res")
                nc.vector.tensor_add(res[:, 0:512], pa[:, :],
                                     bias_t[:, 0:512])
                nc.vector.tensor_add(res[:, 512:768], pb[:, 0:256],
                                     bias_t[:, 512:768])
                nc.sync.dma_start(out=out_d[1, qi * P:(qi + 1) * P, :],
                                  in_=res[:])

        # ---------------- filler queue -----------------------------------
        # items: (due_slot, ready_slot, cols, fn); popped in due order.
        fill = []
        for t in (4, 5):
            fill.append((1, 0, 4608, lambda t=t: v_group(xt_t, vaug1, t)))
        for t in (6, 7):
            fill.append((2, 0, 4608, lambda t=t: v_group(xt_t, vaug1, t)))
        for g in range(2, NDT):
            fill.append((4 * (g - 1) - 1, 0, 6144,
                         lambda g=g: qkT_group(xt_t, g, qT, g)))
            fill.append((4 * (g - 1) - 1, 0, 6144,
                         lambda g=g: qkT_group(xt_t, NDT + g, kT1, g)))
        for t in range(NQT):
            fill.append((22 + t // 2, 0, 4608,
                         lambda t=t: v_group(x2t_t, vaug2, t)))
        for g in range(NDT):
            fill.append((18 + 4 * g, 0, 6144,
                         lambda g=g: qkT_group(x2t_t, NDT + g, kT2, g)))
        for qi in range(6):
            fill.append((28 + 2 * qi, 24, 4608,
                         lambda qi=qi: proj_qi(0, qi)))
        fill.sort(key=lambda it: it[0])

        MINFILL = 2400
        HORIZON = 8

        def pop_fillers(s):
            consumed = 0
            while fill:
                idx = None
                for i, (due, ready, cols, fn) in enumerate(fill):
                    if ready <= s:
                        idx = i
                        break
                if idx is None:
                    break
                due, ready, cols, fn = fill[idx]
                if due <= s or (consumed < MINFILL and due - s <= HORIZON):
                    fill.pop(idx)
                    fn()
                    consumed += cols
                else:
                    break

        # ---------------- lead-in ----------------------------------------
        qkT_group(xt_t, 0, qT, 0)
        qkT_group(xt_t, NDT, kT1, 0)
        part1(0, 0)
        v_group(xt_t, vaug1, 0)
        qkT_group(xt_t, 1, qT, 1)
        part1(0, 1)
        v_group(xt_t, vaug1, 1)
        qkT_group(xt_t, NDT + 1, kT1, 1)
        part1(0, 2)
        v_group(xt_t, vaug1, 2)
        v_group(xt_t, vaug1, 3)
        part1(0, 3)

        # ---------------- unit loop (0..10) -------------------------------
        for u in range(NU - 1):
            for k in range(4):
                s = 4 * u + k
                emit_av(u, k)
                if k == 3:
                    evict_unit(u)
                pop_fillers(s)
                part1(u + 1, k)
            br, g = units[u]
            for bi, gs in enumerate(BATCHES[br]):
                if g == gs[-1]:
                    norm_batch(br, bi)

        # ---------------- unit 11 + tail ----------------------------------
        # No next-unit S work exists, so run the four AV bursts back to back
        # (the g5 row-sum chain starts ~7us earlier), then cover the chain
        # with branch-0 projections qi6/7, SBUF-evicted br1 partials (qi4-6,
        # bias folded in), and held-PSUM br1 partials (qi0-3). Post-scale
        # work is then just the g5 finish matmuls + output blends.
        assert not fill, f"filler queue not drained: {len(fill)} left"
        for k in range(4):
            emit_av(NU - 1, k)
        evict_unit(NU - 1)
        norm_batch(1, 3)
        proj_qi(0, 6)
        proj_qi(0, 7)
        for qi in (4, 5, 6):
            partial_sb(qi)
        partial_S(0)
        partial_S(1)
        partial_O(2)
        partial_O(3)
        for qi in range(4):
            finish_qi(qi)
        for qi in (4, 5, 6):
            finish_sb(qi)
        proj_qi(1, 7)

    n = _split_multiwaits(nc) if split else 0
    _built = (nc, n)
    return _built


def _host_prep(x, x2, qkv_w, proj_w, proj_b):
    """-> list of 8 per-core input maps (bf16 operands, f32 bias)."""
    import ml_dtypes
    bf = lambda a: np.ascontiguousarray(a).astype(ml_dtypes.bfloat16)

    x = np.asarray(x)
    x2 = np.asarray(x2)
    qkv_w = np.asarray(qkv_w)
    # [768 in, 1536 out] -> [128, 12, 6, 128] (partition, col block, i, col)
    wqk = bf(qkv_w[:2 * D].T.reshape(NDT, P, 2 * NDT, P)
             .transpose(1, 2, 0, 3))
    wv = bf(qkv_w[2 * D:].T.reshape(NDT, P, D).transpose(1, 0, 2))
    wp = bf(np.asarray(proj_w).T.reshape(NDT, P, D).transpose(1, 0, 2))
    bias = bf(np.broadcast_to(np.asarray(proj_b, dtype=np.float32),
                              (P, D)))
    maps = []
    for c in range(B):
        xt = bf(x[c].T.reshape(NDT, P, N).transpose(1, 0, 2))
        x2t = bf(x2[c].T.reshape(NDT, P, N).transpose(1, 0, 2))
        maps.append({
            "xt": xt, "x2t": x2t,
            "wqk": wqk, "wv": wv, "wp": wp, "bias": bias,
        })
    return maps


def kernel(x, x2, qkv_w, proj_w, proj_b, trace=False, tmpdir=None):
    nc, _ = _build()
    from concourse.bass_utils import run_bass_kernel_spmd
    in_maps = _host_prep(x, x2, qkv_w, proj_w, proj_b)
    res = run_bass_kernel_spmd(nc, in_maps, list(range(B)), trace=trace,
                               tmpdir=tmpdir)
    kernel.last_exec_time_ns = res.exec_time_ns
    out = np.stack([np.asarray(res.results[c]["out"]).astype(np.float32)
                    for c in range(B)])  # [B,2,N,D]
    out1 = np.ascontiguousarray(out[:, 0])
    out2 = np.ascontiguousarray(out[:, 1])
    return (out1, out2)


kernel.last_exec_time_ns = None
